# revision 7
# baseline (speedup 1.0000x reference)
"""Trainium2 Bass kernel for nn_BiMap: P = Wo^T X Wo (QR-orthonormalized W),
L = channel-selection objective with affine-invariant SPD distances.

Eigendecomposition-free device algorithm (all batched 64x64 matmuls):
  - inverse Cholesky U = R^-1 of G = W^T W via flat serial Cholesky (tiny)
    + Neumann-product triangular inverse; Wo = W U.
  - logm(X), X^-1/2 via residual-form Newton-Schulz sqrt chains
    (inverse scaling-squaring) with fixed global stage scales.
  - dis_ij = ||logm(isq_i X_j isq_i)||_F for i<j only (symmetry).
  - P = (X Wo)^T Wo congruences; L = 2*sum_{i<j} (sdis*Wo^T diff)^T(...)
    accumulated in PSUM per chunk.

Sharding: data-parallel over batch B=32 across 8 cores (4 samples/core),
W replicated. Full inputs in, full outputs out.
"""

import math
import numpy as np

import concourse.bass as bass
import concourse.bacc as bacc
import concourse.mybir as mybir
import concourse.tile as tile
from concourse.bass_utils import run_bass_kernel_spmd

DT = mybir.dt.float32
F32 = np.float32
OP = mybir.AluOpType
AF = mybir.ActivationFunctionType

# problem shape (hardcoded per harness contract)
B, C, NI, NO, M = 32, 16, 64, 32, 4
NCORES = 8
BL = B // NCORES  # 4 local samples per core

PAIRS = [(i, j) for i in range(C) for j in range(C) if i < j]  # 120
NCHUNK = len(PAIRS) // 8  # 15 chunks of 8 pairs

# Newton-Schulz / log-chain schedules (validated in f32 prototype:
# P rel err ~1e-6, L rel err ~8e-6 vs eigh reference).
XA0 = 5.0
X_ITERS = [12, 7, 5, 4, 4]
X_TAYLOR = 8
PA0 = 250.0  # on Mp_raw = Z X Z = XA0 * (isq X isq)
P_ITERS = [14, 8, 6, 4, 4]
P_TAYLOR = 8
CX = math.log(XA0)
CP = math.log(PA0) - math.log(XA0)


def _consts():
    """Constant tensors shipped as extra kernel inputs (identical per core)."""
    I64 = np.eye(64, dtype=F32)
    I2 = np.concatenate([I64, I64], axis=0)  # [128,64]
    c = {}
    c["cI2W"] = np.tile(I2, (1, 4))  # [128,256]
    c["cHI2W"] = 0.5 * c["cI2W"]
    c["cCI2XW"] = F32(CX) * c["cI2W"]
    c["cCI2PW"] = F32(CP) * c["cI2W"]
    c["cCOPY2"] = np.concatenate([np.concatenate([I64, I64], 1)] * 2, 0)  # [128,128]
    c["cI128"] = np.eye(128, dtype=F32)
    sel = np.zeros((128, 32), dtype=F32)
    sel[:64, 0] = 1.0
    sel[64:, 1] = 1.0
    c["cSEL"] = sel
    hm = np.zeros((2, 512), dtype=F32)
    hm[0, :256] = 1.0
    hm[1, 256:] = 1.0
    c["cHMASK"] = hm
    c["cONES2"] = np.ones((2, 64), dtype=F32)
    c["cI32"] = np.eye(32, dtype=F32)
    c["cUM32"] = np.triu(np.ones((32, 32), dtype=F32), 1)
    c["cONES1_32"] = np.ones((1, 32), dtype=F32)
    return {k: np.ascontiguousarray(v.astype(F32)) for k, v in c.items()}


def build_program(bl=BL):
    nc = bacc.Bacc("TRN2", target_bir_lowering=False, debug=False)

    Xd = nc.dram_tensor("X", [bl, C, NI, NI], DT, kind="ExternalInput").ap()
    Wd = nc.dram_tensor("W", [M, NI, NO], DT, kind="ExternalInput").ap()
    cnp = _consts()
    cd = {
        k: nc.dram_tensor(k, list(v.shape), DT, kind="ExternalInput").ap()
        for k, v in cnp.items()
    }
    Pd = nc.dram_tensor("P", [bl, C, M, NO, NO], DT, kind="ExternalOutput").ap()
    Ld = nc.dram_tensor("L", [bl, M, NI, NI], DT, kind="ExternalOutput").ap()

    Xr = Xd.rearrange("b c x y -> (b c x) y")  # [bl*C*64, 64]
    Xw_view = Xr.rearrange("(t p) y -> p t y", p=128)  # [128, bl*8, 64]
    Pv = Pd.rearrange("b c m n o -> (b c) (m n) o")  # [bl*C, 128, 32]
    Lv = Ld.rearrange("b m x y -> (b m x) y")  # [bl*4*64, 64]

    with tile.TileContext(nc) as tc:
        _build_tile(tc, nc, bl, Xw_view, Wd, cd, Pv, Lv)
    nc.compile()
    return nc


def _build_tile(tc, nc, bl, Xw_view, Wd, cd, Pv, Lv):
    from contextlib import ExitStack

    ctx = ExitStack()
    with ctx:
        consts = ctx.enter_context(tc.tile_pool(name="consts", bufs=1))
        wprep = ctx.enter_context(tc.tile_pool(name="wprep", bufs=1))
        wsmall = ctx.enter_context(tc.tile_pool(name="wsmall", bufs=3))
        xw_pool = ctx.enter_context(tc.tile_pool(name="xw", bufs=3))
        chain = ctx.enter_context(tc.tile_pool(name="chain", bufs=3))
        dupz = ctx.enter_context(tc.tile_pool(name="dupz", bufs=18))
        dupx = ctx.enter_context(tc.tile_pool(name="dupx", bufs=18))
        dupl = ctx.enter_context(tc.tile_pool(name="dupl", bufs=18))
        lphase = ctx.enter_context(tc.tile_pool(name="lphase", bufs=2))
        misc = ctx.enter_context(tc.tile_pool(name="misc", bufs=3))
        dram = ctx.enter_context(tc.tile_pool(name="dram", bufs=1, space="DRAM"))
        ps_chain = ctx.enter_context(
            tc.tile_pool(name="ps_chain", bufs=2, space="PSUM")
        )
        ps_big = ctx.enter_context(tc.tile_pool(name="ps_big", bufs=1, space="PSUM"))
        ps_small = ctx.enter_context(
            tc.tile_pool(name="ps_small", bufs=3, space="PSUM")
        )

        # ---- load constants ----
        ct = {}
        for k, ap in cd.items():
            t = consts.tile(list(ap.shape), DT, tag=k)
            nc.sync.dma_start(t[:], ap)
            ct[k] = t
        I2W = ct["cI2W"]
        HI2W = ct["cHI2W"]

        def octet_mm(ps, lhs, rhs, tag_unused=None):
            """8 matmuls: per mtile t (4) and half h (2), 64x64x64 into ps."""
            for t in range(4):
                for h in range(2):
                    sl = (slice(64 * h, 64 * h + 64), slice(64 * t, 64 * t + 64))
                    nc.tensor.matmul(
                        ps[sl],
                        lhs[sl],
                        rhs[sl],
                        start=True,
                        stop=True,
                        tile_position=(64 * h, 64 * h),
                    )

        # =========== W prep: U_m, Wo_all ===========
        Wo_all = consts.tile([64, 128], DT, tag="Wo_all")  # [x, (m,o)]
        dscr = dram.tile([4, 32, 32], DT, tag="dscr")
        WmT_tiles = []
        Gflat = wprep.tile([4, 1024], DT, tag="Gflat")
        for m in range(M):
            wm = wprep.tile([64, 32], DT, tag=f"wm{m}")
            nc.sync.dma_start(wm[:], Wd[m])
            pt = ps_small.tile([32, 64], DT, tag="sps")
            nc.tensor.transpose(pt[:], wm[:], ct["cI128"][0:64, 0:64])
            wmT = wprep.tile([32, 64], DT, tag=f"wmT{m}")
            nc.vector.tensor_copy(wmT[:], pt[:])
            WmT_tiles.append(wmT)
            gp = ps_small.tile([32, 32], DT, tag="sps")
            nc.tensor.matmul(gp[:], wm[:], wm[:], start=True, stop=True)
            gsb = wsmall.tile([32, 32], DT, tag="gsb")
            nc.vector.tensor_copy(gsb[:], gp[:])
            nc.sync.dma_start(dscr[m], gsb[:])
            nc.sync.dma_start(
                Gflat[m : m + 1, :], dscr[m].rearrange("r c -> (r c)")[None, :]
            )

        # ---- flat Cholesky on [4, 1024] (batched over m on partitions) ----
        rd = wprep.tile([4, 1], DT, tag="rdiag")
        outer = wprep.tile([4, 31 * 31], DT, tag="outer")
        for j in range(32):
            djj = Gflat[:, 33 * j : 33 * j + 1]
            nc.vector.reciprocal(rd[:], djj)
            nc.scalar.activation(rd[:], rd[:], AF.Sqrt)  # 1/sqrt(g_jj)
            nc.vector.tensor_scalar_mul(
                Gflat[:, 32 * j + j : 32 * j + 32],
                Gflat[:, 32 * j + j : 32 * j + 32],
                rd[:],
            )
            if j == 31:
                break
            nrem = 31 - j
            base = Gflat[:]
            t0, off = base.tensor, base.offset
            rowb = bass.AP(
                tensor=t0,
                offset=off + 32 * j + (j + 1),
                ap=[base.ap[0], [0, nrem], [1, nrem]],
            )
            colb = bass.AP(
                tensor=t0,
                offset=off + 32 * j + (j + 1),
                ap=[base.ap[0], [1, nrem], [0, nrem]],
            )
            sub = bass.AP(
                tensor=t0,
                offset=off + 33 * (j + 1),
                ap=[base.ap[0], [32, nrem], [1, nrem]],
            )
            oa = outer[:]
            op = bass.AP(
                tensor=oa.tensor,
                offset=oa.offset,
                ap=[oa.ap[0], [31, nrem], [1, nrem]],
            )
            nc.vector.tensor_tensor(op, colb, rowb, OP.mult)
            nc.vector.tensor_tensor(sub, sub, op, OP.subtract)

        # ---- per-m: N = strict_upper(D^-1 R), U via Neumann product ----
        for m in range(M):
            nc.sync.dma_start(
                dscr[m].rearrange("r c -> (r c)")[None, :], Gflat[m : m + 1, :]
            )
            R = wsmall.tile([32, 32], DT, tag="R")
            nc.sync.dma_start(R[:], dscr[m])
            dg = wsmall.tile([32, 1], DT, tag="dg")
            dgscr = wsmall.tile([32, 32], DT, tag="dgscr")
            dmul = wsmall.tile([32, 32], DT, tag="dmul")
            nc.vector.tensor_tensor(dmul[:], R[:], ct["cI32"][:], OP.mult)
            nc.scalar.activation(dgscr[:], dmul[:], AF.Copy, accum_out=dg[:])
            rdg = wsmall.tile([32, 1], DT, tag="rdg")
            nc.vector.reciprocal(rdg[:], dg[:])
            N = wsmall.tile([32, 32], DT, tag="N")
            nc.vector.tensor_scalar_mul(N[:], R[:], rdg[:])
            nc.vector.tensor_tensor(N[:], N[:], ct["cUM32"][:], OP.mult)
            ptn = ps_small.tile([32, 32], DT, tag="sps")
            nc.tensor.transpose(ptn[:], N[:], ct["cI128"][0:32, 0:32])
            NT = wsmall.tile([32, 32], DT, tag="NT")
            nc.vector.tensor_copy(NT[:], ptn[:])
            # forward & transposed powers N^(2^k)
            Ak, AkT = N, NT
            AT_list = [NT]
            for lev in range(3):  # N^2, N^4, N^8 (both); N^16 fwd only
                p1 = ps_small.tile([32, 32], DT, tag="sps")
                nc.tensor.matmul(p1[:], AkT[:], Ak[:], start=True, stop=True)
                A2 = wsmall.tile([32, 32], DT, tag=f"A2_{lev}")
                nc.vector.tensor_copy(A2[:], p1[:])
                p2 = ps_small.tile([32, 32], DT, tag="sps")
                nc.tensor.matmul(p2[:], Ak[:], AkT[:], start=True, stop=True)
                A2T = wsmall.tile([32, 32], DT, tag=f"A2T_{lev}")
                nc.vector.tensor_copy(A2T[:], p2[:])
                AT_list.append(A2T)
                Ak, AkT = A2, A2T
            p16 = ps_small.tile([32, 32], DT, tag="sps")
            nc.tensor.matmul(p16[:], AkT[:], Ak[:], start=True, stop=True)  # N^16
            # Q = I + N^16
            Q = wsmall.tile([32, 32], DT, tag="Q")
            nc.vector.scalar_tensor_tensor(
                Q[:], p16[:], 1.0, ct["cI32"][:], OP.mult, OP.add
            )
            # Q <- (I + N^{2^k}) Q  for k=3(8),2(4),1(2); then (I - N) Q
            for k in (3, 2, 1):
                lhs = wsmall.tile([32, 32], DT, tag="lhsIk")
                nc.vector.tensor_tensor(lhs[:], ct["cI32"][:], AT_list[k][:], OP.add)
                pq = ps_small.tile([32, 32], DT, tag="sps")
                nc.tensor.matmul(pq[:], lhs[:], Q[:], start=True, stop=True)
                Q = wsmall.tile([32, 32], DT, tag="Q")
                nc.vector.tensor_copy(Q[:], pq[:])
            lhs0 = wsmall.tile([32, 32], DT, tag="lhsIk")
            nc.vector.tensor_tensor(lhs0[:], ct["cI32"][:], AT_list[0][:], OP.subtract)
            pq = ps_small.tile([32, 32], DT, tag="sps")
            nc.tensor.matmul(pq[:], lhs0[:], Q[:], start=True, stop=True)
            Q = wsmall.tile([32, 32], DT, tag="Q")
            nc.vector.tensor_copy(Q[:], pq[:])
            # column scale by 1/diag: bcast tile via two matmuls
            prow = ps_small.tile([1, 32], DT, tag="sps")
            nc.tensor.matmul(prow[:], rdg[:], ct["cI32"][:], start=True, stop=True)
            rowsb = wsmall.tile([1, 32], DT, tag="rowsb")
            nc.vector.tensor_copy(rowsb[:], prow[:])
            pcs = ps_small.tile([32, 32], DT, tag="sps")
            nc.tensor.matmul(
                pcs[:], ct["cONES1_32"][:], rowsb[:], start=True, stop=True
            )
            U = wsmall.tile([32, 32], DT, tag="U")
            nc.vector.tensor_tensor(U[:], Q[:], pcs[:], OP.mult)
            # Wo_m = W_m U = mm(lhsT=WmT, rhs=U)
            pw = ps_small.tile([64, 32], DT, tag="sps")
            nc.tensor.matmul(pw[:], WmT_tiles[m][:], U[:], start=True, stop=True)
            nc.vector.tensor_copy(Wo_all[:, 32 * m : 32 * m + 32], pw[:])

        # =========== NS log chain on one octet (wide [128, 256]) ===========
        def ns_log_chain(Yw, Hw, iters_list, taylor, ci2w, z_out=None):
            Zw = None
            for s, nit in enumerate(iters_list):
                if s > 0:
                    Hn = chain.tile([128, 256], DT, tag="Hw")
                    nc.vector.scalar_tensor_tensor(
                        Hn[:], Yw[:], -0.5, HI2W[:], OP.mult, OP.add
                    )
                    Hw = Hn
                for k in range(nit):
                    psy = ps_chain.tile([128, 256], DT, tag="cps")
                    octet_mm(psy, Yw, Hw)
                    Yn = chain.tile([128, 256], DT, tag="Yw")
                    nc.vector.tensor_tensor(Yn[:], Yw[:], psy[:], OP.add)
                    if s == 0 and z_out is not None:
                        if k == 0:
                            Zw = chain.tile([128, 256], DT, tag="Zw")
                            nc.vector.tensor_tensor(Zw[:], Hw[:], I2W[:], OP.add)
                        else:
                            psz = ps_chain.tile([128, 256], DT, tag="cps")
                            octet_mm(psz, Hw, Zw)
                            Zn = chain.tile([128, 256], DT, tag="Zw")
                            nc.vector.tensor_tensor(Zn[:], Zw[:], psz[:], OP.add)
                            Zw = Zn
                    if k < nit - 1:
                        ps2 = ps_chain.tile([128, 256], DT, tag="cps")
                        octet_mm(ps2, Hw, Hw)
                        HH = chain.tile([128, 256], DT, tag="HHw")
                        nc.vector.tensor_copy(HH[:], ps2[:])
                        ps3 = ps_chain.tile([128, 256], DT, tag="cps")
                        octet_mm(ps3, HH, Hw)
                        Hn = chain.tile([128, 256], DT, tag="Hw")
                        nc.vector.scalar_tensor_tensor(
                            Hn[:], HH[:], 1.5, ps3[:], OP.mult, OP.add
                        )
                        Hw = Hn
                    Yw = Yn
                if s == 0 and z_out is not None:
                    nc.vector.tensor_copy(z_out[:], Zw[:])
            # Taylor
            E = chain.tile([128, 256], DT, tag="Ew")
            nc.vector.tensor_tensor(E[:], Yw[:], I2W[:], OP.subtract)
            T = taylor
            t1 = chain.tile([128, 256], DT, tag="Sw")
            nc.vector.tensor_scalar_mul(t1[:], E[:], 1.0 / T)
            S = chain.tile([128, 256], DT, tag="Sw")
            nc.vector.scalar_tensor_tensor(
                S[:], I2W[:], 1.0 / (T - 1), t1[:], OP.mult, OP.subtract
            )
            for k in range(T - 3, -1, -1):
                pss = ps_chain.tile([128, 256], DT, tag="cps")
                octet_mm(pss, E, S)
                S = chain.tile([128, 256], DT, tag="Sw")
                nc.vector.scalar_tensor_tensor(
                    S[:], I2W[:], 1.0 / (k + 1), pss[:], OP.mult, OP.subtract
                )
            psl = ps_chain.tile([128, 256], DT, tag="cps")
            octet_mm(psl, E, S)
            logw = chain.tile([128, 256], DT, tag="logw")
            nc.vector.scalar_tensor_tensor(
                logw[:], psl[:], 32.0, ci2w[:], OP.mult, OP.add
            )
            return logw

        def dup_matrix(src_ap64, h, tag, pool):
            """src_ap64: [64,64] AP at partition half h -> [128,64] duplicated."""
            ps = ps_small.tile([128, 64], DT, tag="sps")
            nc.tensor.matmul(
                ps[:],
                ct["cCOPY2"][64 * h : 64 * h + 64, :],
                src_ap64,
                start=True,
                stop=True,
            )
            t = pool.tile([128, 64], DT, tag=tag)
            nc.vector.tensor_copy(t[:], ps[:])
            return t

        # =========== per-sample processing ===========
        for b in range(bl):
            Zdup, Xdup, Ldup = {}, {}, {}
            for g in range(2):
                Xw = xw_pool.tile([128, 4, 64], DT, tag="Xw")
                nc.sync.dma_start(
                    Xw[:], Xw_view[:, b * 8 + 4 * g : b * 8 + 4 * g + 4, :]
                )
                Xwf = Xw[:].rearrange("p t y -> p (t y)")
                Y0 = chain.tile([128, 256], DT, tag="Yw")
                nc.vector.tensor_scalar_mul(Y0[:], Xwf, 1.0 / XA0)
                H0 = chain.tile([128, 256], DT, tag="Hw")
                nc.vector.scalar_tensor_tensor(
                    H0[:], Xwf, -0.5 / XA0, HI2W[:], OP.mult, OP.add
                )
                z_out = misc.tile([128, 256], DT, tag="zoct")
                logw = ns_log_chain(
                    Y0, H0, X_ITERS, X_TAYLOR, ct["cCI2XW"], z_out=z_out
                )
                for t in range(4):
                    for h in range(2):
                        ci = 8 * g + 2 * t + h
                        sl = (slice(64 * h, 64 * h + 64), slice(64 * t, 64 * t + 64))
                        Zdup[ci] = dup_matrix(z_out[sl], h, "zdup", dupz)
                        Xdup[ci] = dup_matrix(Xwf[sl], h, "xdup", dupx)
                        Ldup[ci] = dup_matrix(logw[sl], h, "ldup", dupl)

            Lsb = []
            for half in range(2):
                t = misc.tile([128, 64], DT, tag=f"Lsb{half}")
                nc.vector.memset(t[:], 0.0)
                Lsb.append(t)

            for cidx in range(NCHUNK):
                pair8 = PAIRS[8 * cidx : 8 * cidx + 8]
                psf = ps_chain.tile([128, 256], DT, tag="cps")
                for p8, (i, j) in enumerate(pair8):
                    t, h = p8 // 2, p8 % 2
                    sl = (slice(64 * h, 64 * h + 64), slice(64 * t, 64 * t + 64))
                    hs = slice(64 * h, 64 * h + 64)
                    pt2 = ps_small.tile([128, 64], DT, tag="sps")
                    nc.tensor.matmul(
                        pt2[hs, :],
                        Xdup[j][hs, :],
                        Zdup[i][hs, :],
                        start=True,
                        stop=True,
                        tile_position=(64 * h, 64 * h),
                    )
                    t2 = misc.tile([128, 64], DT, tag="t2sb")
                    nc.vector.tensor_copy(t2[hs, :], pt2[hs, :])
                    nc.tensor.matmul(
                        psf[sl],
                        Zdup[i][hs, :],
                        t2[hs, :],
                        start=True,
                        stop=True,
                        tile_position=(64 * h, 64 * h),
                    )
                Yw = chain.tile([128, 256], DT, tag="Yw")
                nc.vector.tensor_scalar_mul(Yw[:], psf[:], 1.0 / PA0)
                Hw = chain.tile([128, 256], DT, tag="Hw")
                nc.vector.scalar_tensor_tensor(
                    Hw[:], psf[:], -0.5 / PA0, HI2W[:], OP.mult, OP.add
                )
                logw = ns_log_chain(Yw, Hw, P_ITERS, P_TAYLOR, ct["cCI2PW"])
                partials = misc.tile([128, 4], DT, tag="partials")
                scr = chain.tile([128, 256], DT, tag="scr")
                for t in range(4):
                    cs = slice(64 * t, 64 * t + 64)
                    nc.scalar.activation(
                        scr[:, cs],
                        logw[:, cs],
                        AF.Square,
                        accum_out=partials[:, t : t + 1],
                    )
                pd2 = ps_small.tile([32, 4], DT, tag="sps")
                nc.tensor.matmul(
                    pd2[:], ct["cSEL"][:], partials[:], start=True, stop=True
                )
                sdis = misc.tile([32, 4], DT, tag="sdis")
                nc.scalar.activation(sdis[:], pd2[:], AF.Sqrt)
                nc.scalar.activation(sdis[:], sdis[:], AF.Sqrt)
                sa = sdis[0:2, :]
                sdis_b = bass.AP(
                    tensor=sa.tensor,
                    offset=sa.offset,
                    ap=[sa.ap[0], [0, 2], [1, 4], [0, 64]],
                )
                Bexp = lphase.tile([2, 512], DT, tag="Bexp")
                hmask = ct["cHMASK"][:].rearrange("p (d t y) -> p d t y", d=2, t=4)
                nc.vector.tensor_tensor(Bexp[:].rearrange("p (d t y) -> p d t y", d=2, t=4), sdis_b, hmask, OP.mult)
                pse = ps_big.tile([64, 512], DT, tag="bps")
                nc.tensor.matmul(
                    pse[:], ct["cONES2"][:], Bexp[:], start=True, stop=True
                )
                Dt = lphase.tile([64, 512], DT, tag="Dt")
                for blk in range(8):
                    p8 = 2 * blk if blk < 4 else 2 * (blk - 4) + 1
                    i, j = pair8[p8]
                    nc.vector.tensor_tensor(
                        Dt[:, 64 * blk : 64 * blk + 64],
                        Ldup[i][0:64, :],
                        Ldup[j][0:64, :],
                        OP.subtract,
                    )
                nc.vector.tensor_tensor(Dt[:], Dt[:], pse[:], OP.mult)
                psv = ps_big.tile([128, 512], DT, tag="bps")
                nc.tensor.matmul(psv[:], Wo_all[:], Dt[:], start=True, stop=True)
                Vsb = lphase.tile([128, 512], DT, tag="Vsb")
                nc.vector.tensor_copy(Vsb[:], psv[:])
                plg = [
                    ps_small.tile([128, 64], DT, tag="sps", name=f"plg{_h}")
                    for _h in range(2)
                ]
                for m in range(M):
                    half, colp = m // 2, (m % 2) * 64
                    for blk in range(8):
                        nc.tensor.matmul(
                            plg[half][colp : colp + 64, :],
                            Vsb[32 * m : 32 * m + 32, 64 * blk : 64 * blk + 64],
                            Vsb[32 * m : 32 * m + 32, 64 * blk : 64 * blk + 64],
                            start=(blk == 0),
                            stop=(blk == 7),
                            tile_position=(32 * m, colp),
                        )
                for half in range(2):
                    nc.vector.tensor_tensor(
                        Lsb[half][:], Lsb[half][:], plg[half][:], OP.add
                    )

            for half in range(2):
                lo = misc.tile([128, 64], DT, tag="lout")
                nc.vector.tensor_scalar_mul(lo[:], Lsb[half][:], 2.0)
                nc.sync.dma_start(
                    Lv[256 * b + 128 * half : 256 * b + 128 * half + 128, :], lo[:]
                )

            for c in range(C):
                pxw = ps_small.tile([64, 128], DT, tag="sps")
                nc.tensor.matmul(
                    pxw[:], Xdup[c][0:64, :], Wo_all[:], start=True, stop=True
                )
                xwo = misc.tile([64, 128], DT, tag="xwo_sb")
                nc.vector.tensor_copy(xwo[:], pxw[:])
                pp = ps_small.tile([128, 32], DT, tag="sps")
                for m in range(M):
                    nc.tensor.matmul(
                        pp[32 * m : 32 * m + 32, :],
                        xwo[:, 32 * m : 32 * m + 32],
                        Wo_all[:, 32 * m : 32 * m + 32],
                        start=True,
                        stop=True,
                        tile_position=(0, 32 * m),
                    )
                psb = misc.tile([128, 32], DT, tag="p_sb")
                nc.vector.tensor_copy(psb[:], pp[:])
                nc.sync.dma_start(Pv[b * C + c], psb[:])


_NC_CACHE = {}


def kernel(X, W):
    X = np.ascontiguousarray(X, dtype=np.float32)
    W = np.ascontiguousarray(W, dtype=np.float32)
    if "nc" not in _NC_CACHE:
        _NC_CACHE["nc"] = build_program(BL)
    nc = _NC_CACHE["nc"]
    cnp = _consts()
    in_maps = []
    for cid in range(NCORES):
        m = {"X": np.ascontiguousarray(X[cid * BL : (cid + 1) * BL]), "W": W}
        m.update(cnp)
        in_maps.append(m)
    res = run_bass_kernel_spmd(nc, in_maps, core_ids=list(range(NCORES)))
    P = np.concatenate([r["P"] for r in res.results], axis=0)
    L = np.concatenate([r["L"] for r in res.results], axis=0)
    P = P.reshape(B, C * M, NO, NO)
    return P, L
